# revision 1
# baseline (speedup 1.0000x reference)
# Trainium2 Bass kernel for nn_ConvRec (DynamicConv sequence model + sampled loss).
#
# Sharding: data-parallel over batch B=512 -> 64 sequences per core on 8 cores.
# Each core computes partial (masked loss sum, mask count); host combines.
#
# Per-core design:
#   feature-major x: (C=128 partitions, NF = SB*204 cols) with a 4-col zero gap
#   before each sequence (conv shifts are free-dim offsets).
#   LayerNorm: mean via PE ones-matmul broadcast (fp32r); variance row (M=1)
#   DMA'd into a compact (16, 816) tile where rsqrt runs via tiny ACT Ln/Exp;
#   rsqrt broadcast back via a 1-row PE matmul; fused DVE applies.
#   DynamicConv: unnormalized exp weights (bf16) -> PE broadcast matmuls (S_k)
#   -> bf16 DVE multiply-accumulate over 5 taps; softmax denominator folded in
#   at the end via DVE reciprocal + per-head broadcast matmul.
#   All gamma/beta folded into weights/biases on the host.
#   Head: PE transposes to token-major, per-column indirect-DMA gathers of
#   pos/neg embedding rows, DVE dot products, masked reduction to 2 scalars.
import numpy as np

L, H, K, C, F, B, T, V = 2, 4, 5, 128, 512, 512, 200, 100000
NCORES = 8
PAD = 4
SEQW = T + PAD            # 204
CH = 2 * SEQW             # 408 cols per chunk (= 2 sequences)
EPS = 1e-5

_CACHE = {}


def _make_tc_class():
    """TileContext whose exit barrier splits sem waits across nops — the
    installed walrus rejects >1 sync wait per instruction."""
    import concourse.tile as tile
    from concourse import mybir
    from concourse.vector_clock import ScopedClock

    class SplitWaitTC(tile.TileContext):
        def _drain_and_barrier(self, tick_clock, wait_clock):
            nc = self.nc
            probe = nc.sync.nop()
            wait_clock.add_sem_waits(
                probe.ins, ScopedClock({None: tick_clock.global_clock}))
            waits = list(probe.ins.sync_info.on_wait)
            probe.ins.sync_info = mybir.SyncInfo(on_wait=[], on_update=[])
            for w in waits:
                wn = nc.sync.nop()
                wn.ins.sync_info = mybir.SyncInfo(on_wait=[w], on_update=[])
            nc.sync.drain()
            nc.all_engine_barrier()
            assert self.sems is not None
            popped = nc._tile_sem_poison_stack.pop()
            assert popped is self._sem_poison
            nc.clear_and_free_semaphores(list(self.sems.allocated().values()))
            nc.all_engine_barrier()

    return SplitWaitTC


def _hoist_extra_waits(nc):
    """The installed walrus accepts only ONE sync wait per instruction.
    Move surplus waits onto dedicated same-engine nops placed just before
    the owning instruction (engine execution is sequential, so semantics
    are identical)."""
    from concourse import mybir

    plan = {}
    for bb in nc.main_func.blocks:
        for ins in bb.instructions:
            si = ins.sync_info
            if si is not None and len(si.on_wait) > 1:
                plan[ins.name] = ins
    if not plan:
        return
    created = {}
    created_names = set()
    for name, ins in plan.items():
        si = ins.sync_info
        waits = list(si.on_wait)
        nops = []
        for w in waits[1:]:
            bi = nc.engines[ins.engine].nop()
            bi.ins.sync_info = mybir.SyncInfo(on_wait=[w], on_update=[])
            nops.append(bi.ins)
            created_names.add(bi.ins.name)
        ins.sync_info = mybir.SyncInfo(on_wait=waits[:1],
                                       on_update=list(si.on_update))
        created[name] = nops
    for bb in nc.main_func.blocks:
        new = []
        for ins in bb.instructions:
            if ins.name in created_names:
                continue
            if ins.name in created:
                new.extend(created[ins.name])
            new.append(ins)
        bb.instructions = new


def _build(SB, debug_taps=()):
    """Emit the Bass program for SB sequences per core."""
    import concourse.bass as bass
    import concourse.tile as tile
    from concourse import mybir
    from concourse.masks import make_identity

    f32 = mybir.dt.float32
    f32r = mybir.dt.float32r
    bf16 = mybir.dt.bfloat16
    i32 = mybir.dt.int32
    Alu = mybir.AluOpType
    Act = mybir.ActivationFunctionType

    NF = SB * SEQW
    NCH = NF // CH            # chunks of 408 cols (2 seqs each)
    GS = min(4, SB)           # head group: seqs per group
    NG = SB // GS

    def r32(ap):
        return ap.bitcast(f32r)

    nc = bass.Bass()

    # ---- DRAM I/O ----
    emb = nc.dram_tensor("item_emb", [V + 1, C], f32, kind="ExternalInput")
    NTC = SB * SEQW // 128
    seqw_d = nc.dram_tensor("seqw", [128, NTC], i32, kind="ExternalInput")
    posw_d = nc.dram_tensor("posw", [128, NTC], i32, kind="ExternalInput")
    negw_d = nc.dram_tensor("negw", [128, NTC], i32, kind="ExternalInput")
    cw_d = nc.dram_tensor("cw_all", [C, L * 20], bf16, kind="ExternalInput")
    cb_d = nc.dram_tensor("cb_all", [20, L], f32, kind="ExternalInput")
    sk_d = nc.dram_tensor("sk_all", [20, K * C], bf16, kind="ExternalInput")
    sblk_d = nc.dram_tensor("sblk", [20, H], bf16, kind="ExternalInput")
    sh128_d = nc.dram_tensor("sh128", [H, C], f32, kind="ExternalInput")
    fc1_d = nc.dram_tensor("fc1_all", [C, L * F], bf16, kind="ExternalInput")
    fc1b_d = nc.dram_tensor("fc1b_all", [C, L * 4], f32, kind="ExternalInput")
    fc2_d = nc.dram_tensor("fc2_all", [C, L * F], bf16, kind="ExternalInput")
    lng_d = nc.dram_tensor("lng_all", [C, L], f32, kind="ExternalInput")
    sln_d = nc.dram_tensor("sln", [C, 2], f32, kind="ExternalInput")
    pend_d = nc.dram_tensor("pendc_all", [C, L + 2], f32, kind="ExternalInput")
    onesC_d = nc.dram_tensor("onesC", [C, C], f32r, kind="ExternalInput")
    ident_d = nc.dram_tensor("ident", [C, C], f32r, kind="ExternalInput")
    zeros4_d = nc.dram_tensor("zeros4", [C, PAD], f32r, kind="ExternalInput")
    out_d = nc.dram_tensor("out", [1, 2], f32, kind="ExternalOutput")

    TC = _make_tc_class()
    with TC(nc) as tc:
        import contextlib
        ctx = contextlib.ExitStack()
        with ctx:
            cpool = ctx.enter_context(tc.tile_pool(name="consts", bufs=1))
            big = ctx.enter_context(tc.tile_pool(name="big", bufs=1))

            # ---- constant / weight tiles ----
            onesC = cpool.tile([128, 128], f32r, tag="onesC")
            nc.sync.dma_start(onesC[:], onesC_d[:])
            onesCb = cpool.tile([128, 128], bf16, tag="onesCb")
            nc.gpsimd.memset(onesCb[:], 1.0 / C)
            ones1 = cpool.tile([128, 1], f32, tag="ones1")
            nc.gpsimd.memset(ones1[:], 1.0)
            onesr = cpool.tile([1, 128], f32, tag="onesr")
            nc.gpsimd.memset(onesr[:], 1.0)
            ident = cpool.tile([128, 128], f32r, tag="ident")
            nc.sync.dma_start(ident[:], ident_d[:])
            epsv = cpool.tile([128, 1], f32, tag="epsv")
            nc.gpsimd.memset(epsv[:], EPS)
            zerov = cpool.tile([128, 1], f32, tag="zerov")
            nc.gpsimd.memset(zerov[:], 0.0)
            eps24 = cpool.tile([128, 1], f32, tag="eps24")
            nc.gpsimd.memset(eps24[:], 1e-24)
            one24 = cpool.tile([128, 1], f32, tag="one24")
            nc.gpsimd.memset(one24[:], 1.0 + 1e-24)

            def ld(tag, dram, shape, dt):
                t = cpool.tile(shape, dt, tag=tag)
                nc.sync.dma_start(t[:], dram[:])
                return t

            cw_sb = ld("cw", cw_d, [C, L * 20], bf16)
            cb_sb = ld("cb", cb_d, [20, L], f32)
            sk_sb = ld("sk", sk_d, [20, K * C], bf16)
            sblk_sb = ld("sblk", sblk_d, [20, H], bf16)
            sh128_sb = ld("sh128", sh128_d, [H, C], f32)
            fc1_sb = ld("fc1", fc1_d, [C, L * F], bf16)
            fc1b_sb = ld("fc1b", fc1b_d, [C, L * 4], f32)
            fc2_sb = ld("fc2", fc2_d, [C, L * F], bf16)
            lng_sb = ld("lng", lng_d, [C, L], f32)
            sln_sb = ld("sln", sln_d, [C, 2], f32)
            pend_sb = ld("pend", pend_d, [C, L + 2], f32)
            seqw_sb = ld("seqw", seqw_d, [128, NTC], i32)
            posw_sb = ld("posw", posw_d, [128, NTC], i32)
            negw_sb = ld("negw", negw_d, [128, NTC], i32)

            # ---- big persistent buffers ----
            xT = big.tile([128, NF], f32r, tag="xT")    # residual / z stream
            xC = big.tile([128, NF], f32r, tag="xC")    # centered / v stream

            dbg_bufs = {}
            for name in debug_taps:
                dbg_bufs[name] = nc.dram_tensor(
                    f"dbg_{name}", [128, NF], f32, kind="ExternalOutput")

            def tap(name, buf=None):
                if name in dbg_bufs:
                    nc.sync.dma_start(dbg_bufs[name][:],
                                      (xT if buf is None else buf)[:, :])

            # ================= input: gather + transpose to feature-major ====
            # Slot j covers gap-indexed columns [128j, 128j+128); gap rows
            # gather item_emb[0] (zeros). A strided memset re-zeros the gap
            # columns defensively after the fills.
            with tc.tile_pool(name="inp", bufs=8) as gp, \
                 tc.tile_pool(name="inps", bufs=4, space="PSUM") as gps:
                for j in range(NTC):
                    gt = gp.tile([128, C], f32r, tag="g")
                    nc.gpsimd.indirect_dma_start(
                        out=gt[:], out_offset=None, in_=emb[:].bitcast(f32r),
                        in_offset=bass.IndirectOffsetOnAxis(
                            ap=seqw_sb[:, j:j + 1], axis=0))
                    tp = gps.tile([128, 128], f32r, tag="tp")
                    nc.tensor.transpose(tp[:], gt[:], ident[:])
                    nc.scalar.copy(xT[:, j * 128:(j + 1) * 128], tp[:])
            xg = xT[:].rearrange("p (s w) -> p s w", w=SEQW)
            zsrc = zeros4_d[:, None, :].to_broadcast((128, SB, PAD))
            nc.sync.dma_start(xg[:, :, 0:PAD], zsrc)
            tap("inp")

            # ================= layers ====================
            bigl_pool = tc.tile_pool(name="bigl", bufs=1)
            bigl = bigl_pool.__enter__()
            # bf16 normalized-activation buffer (leading PAD cols for shifts)
            xbf = bigl.tile([128, NF + PAD], bf16, tag="xbf")
            scr = bigl.tile([128, NF], bf16, tag="scr")  # LN cent / conv accum

            def layernorm(src, cent, pend_ap, dst, dst_off=0):
                """dst[:, dst_off + i] = (src+pend-mean)*rsqrt(var+eps).
                src is (128,NF) f32; cent (bf16) receives centered values;
                dst is bf16. pend_ap: (128,1) AP of centered pending
                per-channel bias (or 0.0)."""
                assert NCH % 2 == 0
                with tc.tile_pool(name="lnp", bufs=4) as sp, \
                     tc.tile_pool(name="lnps", bufs=3, space="PSUM") as pp:
                    for cp in range(NCH // 2):
                        lnv = sp.tile([128, 2, CH], bf16, tag="lnv")
                        for h2 in range(2):
                            ci = 2 * cp + h2
                            cs = slice(ci * CH, (ci + 1) * CH)
                            mu = pp.tile([128, CH], f32, tag="mu")
                            nc.tensor.matmul(mu[:], onesC[:],
                                             src[:, cs],
                                             start=True, stop=True)
                            nc.vector.scalar_tensor_tensor(
                                out=cent[:, cs], in0=src[:, cs],
                                scalar=pend_ap, in1=mu[:],
                                op0=Alu.add, op1=Alu.subtract)
                            sq = sp.tile([128, CH], bf16, tag="sq")
                            nc.gpsimd.tensor_tensor(out=sq[:], in0=cent[:, cs],
                                                    in1=cent[:, cs],
                                                    op=Alu.mult)
                            var = pp.tile([128, CH], f32, tag="var")
                            nc.tensor.matmul(var[:], onesCb[:], sq[:],
                                             start=True, stop=True)
                            nc.scalar.activation(lnv[:, h2, :], var[:], Act.Ln,
                                                 bias=epsv[:, :1], scale=1.0)
                        rin = sp.tile([128, 2, CH], bf16, tag="rin")
                        nc.scalar.activation(rin[:], lnv[:], Act.Exp,
                                             bias=zerov[:, :1], scale=-0.5)
                        for h2 in range(2):
                            ci = 2 * cp + h2
                            cs = slice(ci * CH, (ci + 1) * CH)
                            d = dst[:, dst_off + ci * CH:dst_off + (ci + 1) * CH]
                            nc.vector.tensor_tensor(out=d, in0=cent[:, cs],
                                                    in1=rin[:, h2, :],
                                                    op=Alu.mult)

            for l in range(L):
                # ---- LN1 (shared pre-norm, raw): xT -> xnr (bf16) in xbf ----
                layernorm(xT, scr, pend_sb[:, l:l + 1], xbf, dst_off=PAD)
                # zero the pad/gap columns of xbf (conv halo reads)
                with tc.tile_pool(name="gz", bufs=1):
                    nc.vector.memset(xbf[:, 0:PAD], 0.0)
                    xv = xbf[:, PAD:].rearrange("p (s w) -> p s w", w=SEQW)
                    nc.vector.memset(xv[:, :, 0:PAD], 0.0)
                tap(f"ln1_{l}")

                # ---- softmax numerators + dynamic conv (unnormalized) ----
                with tc.tile_pool(name="cvp", bufs=4) as sp, \
                     tc.tile_pool(name="cvps1", bufs=2, space="PSUM") as pp1, \
                     tc.tile_pool(name="cvps2", bufs=2, space="PSUM") as pp2:
                    for ci in range(NCH):
                        c0 = ci * CH
                        cs = slice(c0, c0 + CH)
                        lg = pp1.tile([20, CH], f32, tag="lg")
                        nc.tensor.matmul(lg[:], cw_sb[:, 20 * l:20 * (l + 1)],
                                         xbf[:, PAD + c0:PAD + c0 + CH],
                                         start=True, stop=True)
                        Et = sp.tile([20, CH], bf16, tag="Et")
                        nc.scalar.activation(Et[:], lg[:], Act.Exp,
                                             bias=cb_sb[:, l:l + 1], scale=1.0)
                        Dp = pp1.tile([H, CH], f32, tag="Dp")
                        nc.tensor.matmul(Dp[:], sblk_sb[:], Et[:],
                                         start=True, stop=True)
                        rD = sp.tile([H, CH], f32, tag="rD")
                        nc.vector.reciprocal(rD[:], Dp[:])
                        rdb = pp1.tile([128, CH], f32, tag="rdb")
                        nc.tensor.matmul(rdb[:], sh128_sb[:], rD[:],
                                         start=True, stop=True)
                        for k in range(K):
                            wb = pp2.tile([128, CH], f32, tag="wb")
                            nc.tensor.matmul(wb[:], sk_sb[:, 128 * k:128 * (k + 1)],
                                             Et[:], start=True, stop=True)
                            wbb = sp.tile([128, CH], bf16, tag="wbb")
                            if k == 0:
                                nc.vector.tensor_copy(wbb[:], wb[:])
                            else:
                                nc.scalar.copy(wbb[:], wb[:])
                            xs = xbf[:, c0 + k:c0 + k + CH]
                            if k == 0:
                                nc.vector.tensor_tensor(out=scr[:, cs], in0=wbb[:],
                                                        in1=xs, op=Alu.mult)
                            else:
                                mk = sp.tile([128, CH], bf16, tag="mk")
                                eng = nc.gpsimd if k >= 3 else nc.vector
                                eng.tensor_tensor(out=mk[:], in0=wbb[:],
                                                  in1=xs, op=Alu.mult)
                                nc.gpsimd.tensor_tensor(out=scr[:, cs],
                                                        in0=scr[:, cs],
                                                        in1=mk[:], op=Alu.add)
                        # v = sln_g * y * (1/D)   (sln_b folded into LN2 pend)
                        nc.vector.scalar_tensor_tensor(
                            out=xC[:, cs], in0=scr[:, cs],
                            scalar=sln_sb[:, 0:1], in1=rdb[:],
                            op0=Alu.mult, op1=Alu.mult)
                tap(f"v_{l}", xC)

                # ---- LN2 (per-layer, raw): xC -> ynr (bf16) in xbf ----
                layernorm(xC, scr, pend_sb[:, L + 1:L + 2], xbf, dst_off=PAD)
                tap(f"ln2_{l}", xT)

                # ---- FFN + residual ----
                with tc.tile_pool(name="fp", bufs=2) as sp, \
                     tc.tile_pool(name="fps1", bufs=2, space="PSUM") as pp1, \
                     tc.tile_pool(name="fps2", bufs=2, space="PSUM") as pp2:
                    for ci in range(NCH):
                        c0 = ci * CH
                        cs = slice(c0, c0 + CH)
                        ybf = xbf[:, PAD + c0:PAD + c0 + CH]
                        hr = sp.tile([128, 4, CH], bf16, tag="hr")
                        for mc in range(4):
                            hp = pp1.tile([128, CH], f32, tag="hp")
                            nc.tensor.matmul(
                                hp[:], fc1_sb[:, l * F + mc * 128:l * F + (mc + 1) * 128],
                                ybf, start=True, stop=True)
                            if mc < 2:
                                nc.scalar.activation(
                                    hr[:, mc, :], hp[:], Act.Relu,
                                    bias=fc1b_sb[:, 4 * l + mc:4 * l + mc + 1],
                                    scale=1.0)
                            else:
                                nc.vector.tensor_scalar(
                                    out=hr[:, mc, :], in0=hp[:],
                                    scalar1=fc1b_sb[:, 4 * l + mc:4 * l + mc + 1],
                                    scalar2=0.0, op0=Alu.add, op1=Alu.max)
                        fo = pp2.tile([128, CH], f32, tag="fo")
                        for kc in range(4):
                            nc.tensor.matmul(
                                fo[:], fc2_sb[:, l * F + kc * 128:l * F + (kc + 1) * 128],
                                hr[:, kc, :], start=(kc == 0), stop=(kc == 3))
                        nc.vector.scalar_tensor_tensor(
                            out=xT[:, cs], in0=ybf,
                            scalar=lng_sb[:, l:l + 1], in1=fo[:],
                            op0=Alu.mult, op1=Alu.add)
                tap(f"ffn_{l}")

            # ================= final LN + head ====================
            bigl_pool.__exit__(None, None, None)
            zbf_pool = tc.tile_pool(name="zbf", bufs=1)
            zb = zbf_pool.__enter__()
            zbf = zb.tile([128, NF], bf16, tag="zbf")
            zcent = zb.tile([128, NF], bf16, tag="zcent")
            layernorm(xT, zcent, pend_sb[:, L:L + 1], zbf, dst_off=0)
            # z = sln_g * znr + sln_b  (f32, into xT)
            with tc.tile_pool(name="zp", bufs=2):
                for ci in range(NCH):
                    cs = slice(ci * CH, (ci + 1) * CH)
                    nc.vector.tensor_scalar(
                        out=xT[:, cs], in0=zbf[:, cs],
                        scalar1=sln_sb[:, 0:1], scalar2=sln_sb[:, 1:2],
                        op0=Alu.mult, op1=Alu.add)
            zbf_pool.__exit__(None, None, None)
            tap("zfin")

            GS2 = 8
            NG2 = (NTC + GS2 - 1) // GS2
            mcol = big.tile([128, NTC], f32, tag="mcol")
            kcol = big.tile([128, NTC], f32, tag="kcol")
            with tc.tile_pool(name="hd", bufs=2) as hp, \
                 tc.tile_pool(name="hdg", bufs=6) as hg, \
                 tc.tile_pool(name="hdps", bufs=4, space="PSUM") as hps:
                for g in range(NG2):
                    gs = min(GS2, NTC - g * GS2)
                    ztm = hp.tile([128, GS2, C], f32, tag="ztm")
                    pe = hg.tile([128, GS2, C], f32, tag="pe")
                    ne = hg.tile([128, GS2, C], f32, tag="ne")
                    for sl in range(gs):
                        j = g * GS2 + sl
                        tp = hps.tile([128, 128], f32r, tag="tp")
                        nc.tensor.transpose(tp[:],
                                            xT[:, j * 128:(j + 1) * 128],
                                            ident[:])
                        nc.scalar.copy(ztm[:, sl, :], tp[:])
                        nc.gpsimd.indirect_dma_start(
                            out=pe[:, sl, :], out_offset=None, in_=emb[:],
                            in_offset=bass.IndirectOffsetOnAxis(
                                ap=posw_sb[:, j:j + 1], axis=0))
                        nc.gpsimd.indirect_dma_start(
                            out=ne[:, sl, :], out_offset=None, in_=emb[:],
                            in_offset=bass.IndirectOffsetOnAxis(
                                ap=negw_sb[:, j:j + 1], axis=0))
                    if gs < GS2:
                        nc.vector.memset(ztm[:, gs:, :], 0.0)
                        nc.vector.memset(pe[:, gs:, :], 0.0)
                        nc.vector.memset(ne[:, gs:, :], 0.0)

                    prod = hp.tile([128, GS2, C], f32, tag="prod")
                    plog = hp.tile([128, GS2], f32, tag="plog")
                    nc.vector.tensor_tensor(out=prod[:], in0=pe[:], in1=ztm[:],
                                            op=Alu.mult)
                    nc.vector.tensor_reduce(out=plog[:], in_=prod[:],
                                            axis=mybir.AxisListType.X, op=Alu.add)
                    prodn = hp.tile([128, GS2, C], f32, tag="prod")
                    nlog = hp.tile([128, GS2], f32, tag="nlog")
                    nc.vector.tensor_tensor(out=prodn[:], in0=ne[:], in1=ztm[:],
                                            op=Alu.mult)
                    nc.vector.tensor_reduce(out=nlog[:], in_=prodn[:],
                                            axis=mybir.AxisListType.X, op=Alu.add)

                    sp_ = hp.tile([128, GS2], f32, tag="sp")
                    nc.scalar.activation(sp_[:], plog[:], Act.Sigmoid,
                                         bias=zerov[:, :1], scale=1.0)
                    lp = hp.tile([128, GS2], f32, tag="lp")
                    nc.scalar.activation(lp[:], sp_[:], Act.Ln,
                                         bias=eps24[:, :1], scale=1.0)
                    sn = hp.tile([128, GS2], f32, tag="sn")
                    nc.scalar.activation(sn[:], nlog[:], Act.Sigmoid,
                                         bias=zerov[:, :1], scale=1.0)
                    ln2 = hp.tile([128, GS2], f32, tag="ln2")
                    nc.scalar.activation(ln2[:], sn[:], Act.Ln,
                                         bias=one24[:, :1], scale=-1.0)
                    ppos = hp.tile([128, GS2], f32, tag="ppos")
                    nc.vector.scalar_tensor_tensor(
                        out=ppos[:], in0=lp[:], scalar=-1.0, in1=ln2[:],
                        op0=Alu.mult, op1=Alu.subtract)
                    msk = hp.tile([128, GS2], f32, tag="msk")
                    nc.vector.tensor_scalar(
                        out=msk[:, :gs], in0=posw_sb[:, g * GS2:g * GS2 + gs],
                        scalar1=0, scalar2=None, op0=Alu.not_equal)
                    gsl = slice(g * GS2, g * GS2 + gs)
                    nc.vector.tensor_tensor(out=mcol[:, gsl], in0=ppos[:, :gs],
                                            in1=msk[:, :gs], op=Alu.mult)
                    nc.vector.tensor_copy(kcol[:, gsl], msk[:, :gs])

                # final reduction: (128, 2SB) x2 -> (128,2) -> (1,2)
                red = hp.tile([128, 2], f32, tag="red")
                nc.vector.tensor_reduce(out=red[:, 0:1], in_=mcol[:],
                                        axis=mybir.AxisListType.X, op=Alu.add)
                nc.vector.tensor_reduce(out=red[:, 1:2], in_=kcol[:],
                                        axis=mybir.AxisListType.X, op=Alu.add)
                tot = hps.tile([1, 2], f32, tag="tot")
                nc.tensor.matmul(tot[:], ones1[:], red[:], start=True, stop=True)
                osb = hp.tile([1, 2], f32, tag="osb")
                nc.scalar.copy(osb[:], tot[:])
                nc.sync.dma_start(out_d[:], osb[:])

    _hoist_extra_waits(nc)
    return nc


def _prep_host(inputs, SB):
    """Fold norms into weights; wrap index arrays."""
    import ml_dtypes
    bf = ml_dtypes.bfloat16
    item_emb = np.asarray(inputs["item_emb"], np.float32)
    conv_w = np.asarray(inputs["conv_w"], np.float32)
    conv_b = np.asarray(inputs["conv_b"], np.float32)
    ln_g = np.asarray(inputs["ln_g"], np.float32)
    ln_b = np.asarray(inputs["ln_b"], np.float32)
    fc1_w = np.asarray(inputs["fc1_w"], np.float32)
    fc1_b = np.asarray(inputs["fc1_b"], np.float32)
    fc2_w = np.asarray(inputs["fc2_w"], np.float32)
    fc2_b = np.asarray(inputs["fc2_b"], np.float32)
    sln_g = np.asarray(inputs["sln_g"], np.float32)
    sln_b = np.asarray(inputs["sln_b"], np.float32)
    seq = np.asarray(inputs["seq"], np.int32)
    pos = np.asarray(inputs["pos"], np.int32)
    neg = np.asarray(inputs["neg"], np.int32)

    shared = {}
    shared["item_emb"] = item_emb
    cw_eff = np.transpose(conv_w * sln_g[None, :, None], (1, 0, 2))  # (C, L, 20)
    shared["cw_all"] = cw_eff.reshape(C, L * 20).astype(bf)
    cb_eff = conv_b + np.einsum("c,lck->lk", sln_b, conv_w)          # (L, 20)
    shared["cb_all"] = cb_eff.T.copy()                                # (20, L)
    sk = np.zeros((20, K, C), np.float32)
    for k in range(K):
        for c in range(C):
            sk[(c // 32) * K + k, k, c] = 1.0
    shared["sk_all"] = sk.reshape(20, K * C).astype(bf)
    sblk = np.zeros((20, H), np.float32)
    for j in range(20):
        sblk[j, j // K] = 1.0
    shared["sblk"] = sblk.astype(bf)
    sh128 = np.zeros((H, C), np.float32)
    for c in range(C):
        sh128[c // 32, c] = 1.0
    shared["sh128"] = sh128
    fc1_eff = fc1_w * ln_g[:, :, None]                   # (L, C, F)
    shared["fc1_all"] = np.transpose(fc1_eff, (1, 0, 2)).reshape(
        C, L * F).astype(bf)
    fc1b_eff = fc1_b + np.einsum("lc,lcf->lf", ln_b, fc1_w)  # (L, F)
    shared["fc1b_all"] = np.transpose(
        fc1b_eff.reshape(L, 4, 128), (2, 0, 1)).reshape(128, L * 4).copy()
    fc2r = np.transpose(fc2_w.reshape(L, 4, 128, C), (2, 0, 1, 3))
    shared["fc2_all"] = fc2r.reshape(128, L * F).astype(bf)
    shared["lng_all"] = ln_g.T.copy()                     # (C, L)
    shared["sln"] = np.stack([sln_g, sln_b], axis=1)      # (C, 2)
    # pending per-partition constants (centered):
    #  col l (l=0..L-1): LN1 of layer l gets ln_b[l-1]+fc2_b[l-1] (0 for l=0)
    #  col L: final LN gets ln_b[L-1]+fc2_b[L-1]
    #  col L+1: LN2 gets sln_b (the shared-norm beta folded out of v)
    pend = np.zeros((C, L + 2), np.float32)
    for l in range(1, L + 1):
        p = ln_b[l - 1] + fc2_b[l - 1]
        pend[:, l] = p - p.mean()
    pend[:, L + 1] = sln_b - sln_b.mean()
    shared["pendc_all"] = pend
    shared["onesC"] = np.full((C, C), 1.0 / C, np.float32)
    shared["ident"] = np.eye(C, dtype=np.float32)
    shared["zeros4"] = np.zeros((C, PAD), np.float32)

    NTC = SB * SEQW // 128

    def wrap(flat_sb):
        # (SB*T,) -> (128, NTC): slot j, partition p holds the index for
        # gap-indexed column g = j*128+p (0 for the 4 gap columns per seq).
        full = np.zeros(SB * SEQW, np.int32)
        a = flat_sb.reshape(SB, T)
        full = full.reshape(SB, SEQW)
        full[:, PAD:] = a
        return full.reshape(-1).reshape(NTC, 128).T.copy()

    per_core = []
    for c in range(NCORES if SB * NCORES == B else 1):
        s0 = c * SB
        per_core.append({
            "seqw": wrap(seq[s0:s0 + SB].reshape(-1)),
            "posw": wrap(pos[s0:s0 + SB].reshape(-1)),
            "negw": wrap(neg[0, s0 * T:(s0 + SB) * T]),
        })
    return shared, per_core


def kernel(**inputs):
    from concourse.bass_utils import run_bass_kernel_spmd

    SB = B // NCORES
    if "nc" not in _CACHE:
        _CACHE["nc"] = _build(SB)
    nc = _CACHE["nc"]
    shared, per_core = _prep_host(inputs, SB)
    in_maps = [{**shared, **pc} for pc in per_core]
    res = run_bass_kernel_spmd(nc, in_maps, core_ids=list(range(NCORES)))
    num = 0.0
    den = 0.0
    for r in res.results:
        num += float(r["out"][0, 0])
        den += float(r["out"][0, 1])
    return np.float32(num / den)

